# revision 52
# baseline (speedup 1.0000x reference)
"""Trainium2 Bass kernel for a dense transformer block (B=4, T=2048, D=256, H=8).

Sharding: 8 cores, zero collectives. Core c owns batch b=c//2 and query-half
s=c%2. Tokens are host-reordered per core so the core's 1024 query rows are
always tokens 0-1023 (uniform SPMD graph). Each core computes LN1+QKV over all
2048 tokens of its batch (k/v need them), attention for all 8 heads over its
1024 query rows, and the Wo/LN/MLP tail for its rows.

Attention is computed in [k, q] orientation (scores^T) so the softmax
normalizer and ctx both come from PE matmuls against v augmented with a ones
column; the attn output is stored [h, k, q] per core and transposed on the
host during unshard. attn values travel as bf16 (tol 2e-2), the -64*mask bias
is added into PSUM via an fp8 identity matmul.
"""

import math
import os
import sys
from contextlib import ExitStack

import numpy as np

sys.path.insert(0, "/opt/trn_rl_repo")

import concourse.bass as bass
import concourse.bacc as bacc
import concourse.tile as tile
from concourse import mybir
from concourse.masks import make_identity

FP32 = mybir.dt.float32
FP32R = mybir.dt.float32r
BF16 = mybir.dt.bfloat16
FP8 = mybir.dt.float8e5
U8 = mybir.dt.uint8

B, T, D, H, DK = 4, 2048, 256, 8, 32
TQ = 1024          # query rows per core
DH = 4 * D         # 1024 mlp hidden
P = 128
NT = T // P        # 16 token tiles
NTQ = TQ // P      # 8 my-token tiles
KD = D // P        # 2 contraction tiles over D
EPS = 1e-5
MASK_NEG = -64.0   # exp(-64) ~ 1.6e-28 ~ 0; exactly representable in fp8e5
AF = mybir.ActivationFunctionType
ABL = set(os.environ.get("ABLATE", "").split(","))


def r(ap):
    return ap.bitcast(FP32R)


def build_graph():
    nc = bacc.Bacc(None, target_bir_lowering=False)

    xb = nc.declare_dram_parameter("xb", [T, D], FP32, isOutput=False)
    masku = nc.declare_dram_parameter("masku", [T, TQ], U8, isOutput=False)
    wq = nc.declare_dram_parameter("wq", [D, D], BF16, isOutput=False)
    wk = nc.declare_dram_parameter("wk", [D, D], BF16, isOutput=False)
    wvaug = nc.declare_dram_parameter("wvaug", [D, H * 33], BF16, isOutput=False)
    wo = nc.declare_dram_parameter("wo", [D, D], BF16, isOutput=False)
    w1 = nc.declare_dram_parameter("w1", [D, DH], BF16, isOutput=False)
    w2 = nc.declare_dram_parameter("w2", [DH, D], BF16, isOutput=False)
    b1t = nc.declare_dram_parameter("b1t", [P, DH // P], FP32, isOutput=False)
    b2b = nc.declare_dram_parameter("b2b", [P, D], FP32, isOutput=False)
    ln1w = nc.declare_dram_parameter("ln1w", [P, D], FP32, isOutput=False)
    ln1b = nc.declare_dram_parameter("ln1b", [P, D], FP32, isOutput=False)
    ln2w = nc.declare_dram_parameter("ln2w", [P, D], FP32, isOutput=False)
    ln2b = nc.declare_dram_parameter("ln2b", [P, D], FP32, isOutput=False)

    attn_out = nc.declare_dram_parameter("attn_out", [H, T, TQ], BF16, isOutput=True)
    y_out = nc.declare_dram_parameter("y_out", [TQ, D], FP32, isOutput=True)

    with tile.TileContext(nc) as tc, ExitStack() as ctx:
        ctx.enter_context(nc.allow_low_precision(
            reason="bf16 attention/mlp path fits the 2e-2 tolerance"))
        const = ctx.enter_context(tc.tile_pool(name="const", bufs=1))
        resid = ctx.enter_context(tc.tile_pool(name="resid", bufs=1))
        work = ctx.enter_context(tc.tile_pool(name="work", bufs=4))
        stat = ctx.enter_context(tc.tile_pool(name="stat", bufs=8))

        # ---- constants into SBUF ----
        id_f32 = const.tile([P, P], FP32)
        make_identity(nc, id_f32)
        id_f8 = const.tile([P, P], FP8)
        make_identity(nc, id_f8)
        zbias = const.tile([P, 1], FP32)
        nc.vector.memset(zbias, 0.0)
        ebias = const.tile([P, 1], FP32)
        nc.vector.memset(ebias, EPS)
        ones1 = const.tile([1, P], BF16)
        nc.vector.memset(ones1, 1.0)
        id_bf = const.tile([P, P], BF16)
        make_identity(nc, id_bf)

        wq_sb = const.tile([P, KD, D], BF16)
        wk_sb = const.tile([P, KD, D], BF16)
        wva_sb = const.tile([P, KD, H * 33], BF16)
        wo_sb = const.tile([P, KD, D], BF16)
        w1_sb = const.tile([P, KD, DH], BF16)
        for kd in range(KD):
            nc.scalar.dma_start(wq_sb[:, kd, :], wq[kd * P:(kd + 1) * P, :])
            nc.scalar.dma_start(wk_sb[:, kd, :], wk[kd * P:(kd + 1) * P, :])
            nc.scalar.dma_start(wva_sb[:, kd, :], wvaug[kd * P:(kd + 1) * P, :])
            nc.scalar.dma_start(wo_sb[:, kd, :], wo[kd * P:(kd + 1) * P, :])
            nc.scalar.dma_start(w1_sb[:, kd, :], w1[kd * P:(kd + 1) * P, :])
        w2_sb = const.tile([P, DH // P, D], BF16)
        for m in range(DH // P):
            nc.scalar.dma_start(w2_sb[:, m, :], w2[m * P:(m + 1) * P, :])
        b1_sb = const.tile([P, DH // P], FP32)
        nc.sync.dma_start(b1_sb, b1t[:])
        b2_sb = const.tile([P, D], FP32)
        nc.sync.dma_start(b2_sb, b2b[:])
        ln1w_sb = const.tile([P, D], FP32)
        nc.sync.dma_start(ln1w_sb, ln1w[:])
        ln1b_sb = const.tile([P, D], FP32)
        nc.sync.dma_start(ln1b_sb, ln1b[:])
        ln2w_sb = const.tile([P, D], FP32)
        nc.sync.dma_start(ln2w_sb, ln2w[:])
        ln2b_sb = const.tile([P, D], FP32)
        nc.sync.dma_start(ln2b_sb, ln2b[:])

        # ---- residual-stream tiles (live across phases) ----
        xx_mine = resid.tile([P, NTQ, D], BF16)    # LN1 rows 0-1023
        x1 = resid.tile([P, NTQ, D], FP32)
        ctxT = resid.tile([P, 2, TQ], BF16)        # [4h*32, g, q]
        attout = resid.tile([P, NTQ, D], BF16)     # ctx @ Wo, token-major

        def layernorm_tile(dst, src, w_sb=None, b_sb=None):
            """dst = LN(src) over free dim (256), optional affine."""
            st = stat.tile([P, 6], FP32)
            nc.vector.bn_stats(st, src)
            mv = stat.tile([P, 2], FP32)
            nc.vector.bn_aggr(mv, st)
            rstd = stat.tile([P, 1], FP32)
            nc.scalar.activation(rstd, mv[:, 1:2], AF.Sqrt, bias=ebias, scale=1.0)
            nc.vector.reciprocal(rstd, rstd)
            nc.vector.tensor_scalar(
                out=dst, in0=src, scalar1=mv[:, 0:1], scalar2=rstd,
                op0=mybir.AluOpType.subtract, op1=mybir.AluOpType.mult)
            if w_sb is not None:
                nc.vector.tensor_mul(dst, dst, w_sb)
                nc.vector.tensor_add(dst, dst, b_sb)

        with tc.tile_pool(name="attnbuf", bufs=1) as attnbuf:
            kT4 = attnbuf.tile([P, 2, T], BF16)      # [4h*32 dk, g, k]
            qT4 = attnbuf.tile([P, 2, TQ], BF16)
            vaug = attnbuf.tile([P, NT, H * 33], BF16)
            maskneg = attnbuf.tile([P, NT, TQ], FP8)

            # ---- phase A-D: LN1, transpose, QKV projections ----
            with tc.tile_pool(name="pre", bufs=1) as pre, \
                 tc.tile_pool(name="ps_sm", bufs=4, space="PSUM") as ps_sm:
                xxT = pre.tile([P, KD, T], BF16)
                xbig = pre.tile([P, NT, D], FP32)
                mvall = pre.tile([P, NT, 2], FP32)
                rsall = pre.tile([P, NT], FP32)
                for blk in range(2):
                    jlo, jhi = blk * NT // 2, (blk + 1) * NT // 2
                    for jt in range(jlo, jhi):
                        deng = nc.sync if jt % 2 else nc.scalar
                        deng.dma_start(xbig[:, jt, :],
                                       xb[jt * P:(jt + 1) * P, :])
                        st = stat.tile([P, 6], FP32)
                        nc.vector.bn_stats(st, xbig[:, jt, :])
                        nc.vector.bn_aggr(mvall[:, jt, :], st)
                    for jt in range(jlo, jhi):
                        nc.scalar.activation(rsall[:, jt:jt + 1],
                                             mvall[:, jt, 1:2],
                                             AF.Sqrt, bias=ebias, scale=1.0)
                    nc.vector.reciprocal(rsall[:, jlo:jhi], rsall[:, jlo:jhi])
                    for jt in range(jlo, jhi):
                        xxt = work.tile([P, D], BF16)
                        nc.vector.tensor_scalar(
                            out=xxt, in0=xbig[:, jt, :],
                            scalar1=mvall[:, jt, 0:1],
                            scalar2=rsall[:, jt:jt + 1],
                            op0=mybir.AluOpType.subtract,
                            op1=mybir.AluOpType.mult)
                        nc.vector.tensor_mul(xxt, xxt, ln1w_sb)
                        nc.vector.tensor_add(xxt, xxt, ln1b_sb)
                        if jt < NTQ:
                            nc.gpsimd.tensor_copy(xx_mine[:, jt, :], xxt)
                        for kd in range(KD):
                            pt = ps_sm.tile([P, 512], BF16, tag="pt",
                                            name="pt")[:, 0:P]
                            nc.tensor.transpose(
                                pt, xxt[:, kd * P:(kd + 1) * P], id_bf)
                            nc.scalar.copy(xxT[:, kd, jt * P:(jt + 1) * P], pt)

                # qT4/kT4: [4h-group g] rows = wq columns g*128..g*128+127
                for g in range(2):
                    for qc in range(2):
                        pm = ps_sm.tile([P, 512], FP32, tag="ps")
                        for kd in range(KD):
                            nc.tensor.matmul(
                                pm, wq_sb[:, kd, g * P:(g + 1) * P],
                                xxT[:, kd, qc * 512:(qc + 1) * 512],
                                start=(kd == 0), stop=(kd == KD - 1))
                        nc.scalar.copy(qT4[:, g, qc * 512:(qc + 1) * 512], pm)
                    for kc in range(4):
                        pm = ps_sm.tile([P, 512], FP32, tag="ps")
                        for kd in range(KD):
                            nc.tensor.matmul(
                                pm, wk_sb[:, kd, g * P:(g + 1) * P],
                                xxT[:, kd, kc * 512:(kc + 1) * 512],
                                start=(kd == 0), stop=(kd == KD - 1))
                        nc.scalar.copy(kT4[:, g, kc * 512:(kc + 1) * 512], pm)

                # v_aug token-major, bf16, with ones columns at 32::33
                for jt in range(NT):
                    pm = ps_sm.tile([P, 512], FP32, tag="ps", name="pmv")[:, 0:H * 33]
                    for kd in range(KD):
                        nc.tensor.matmul(
                            pm, xxT[:, kd, jt * P:(jt + 1) * P],
                            wva_sb[:, kd, :],
                            start=(kd == 0), stop=(kd == KD - 1))
                    nc.scalar.copy(vaug[:, jt, :], pm)
                    nc.vector.memset(vaug[:, jt, 32::33], 1.0)

                for jt in range(NT):
                    mu = work.tile([P, TQ], U8, tag="masku")
                    nc.scalar.dma_start(mu, masku[jt * P:(jt + 1) * P, :])
                    nc.gpsimd.tensor_scalar_mul(maskneg[:, jt, :], mu, MASK_NEG)

            # ---- phase E: attention ----
            with tc.tile_pool(name="expbuf", bufs=1) as expbuf, \
                 tc.tile_pool(name="ps_s", bufs=6, space="PSUM") as ps_s, \
                 tc.tile_pool(name="ps_ctx", bufs=2, space="PSUM") as ps_ctx:
                QC = 512
                exp_bufs = [
                    expbuf.tile([P, NT, 2, QC], BF16, name=f"exp_sb{i}")
                    for i in range(3)
                ]
                it = 0
                deferred_dmas = []
                deferred_prev = []
                for g in range(2):
                    for qc in range(2):
                        q0 = qc * QC
                        for hp in range(2):
                            exp_sb = exp_bufs[it % 3]
                            it += 1
                            for args in deferred_prev:
                                nc.scalar.dma_start(*args)
                            deferred_prev = deferred_dmas
                            deferred_dmas = []
                            pcs = []
                            for jh in range(2):
                                pc = ps_ctx.tile([33, QC], FP32, tag="pc")
                                pcs.append(pc)
                            for kt in range(NT):
                                for jh in range(2):
                                    h4 = 2 * hp + jh
                                    h = g * 4 + h4
                                    ps1 = ps_s.tile([P, QC], FP32, tag="ps")
                                    nc.tensor.matmul(
                                        ps1,
                                        kT4[32 * h4:32 * (h4 + 1), g,
                                            kt * P:(kt + 1) * P],
                                        qT4[32 * h4:32 * (h4 + 1), g,
                                            q0:q0 + QC],
                                        start=True, stop=False,
                                        tile_position=(32 * h4, 0),
                                        skip_group_check=True)
                                    nc.tensor.matmul(
                                        ps1, id_f8,
                                        maskneg[:, kt, q0:q0 + QC],
                                        start=False, stop=True,
                                        skip_group_check=True)
                                    nc.scalar.activation(
                                        exp_sb[:, kt, jh, :],
                                        ps1, AF.Exp, bias=zbias, scale=1.0)
                                    nc.tensor.matmul(
                                        pcs[jh],
                                        vaug[:, kt, h * 33:(h + 1) * 33],
                                        exp_sb[:, kt, jh, :],
                                        start=(kt == 0),
                                        stop=(kt == NT - 1))
                            for jh in range(2):
                                h4 = 2 * hp + jh
                                h = g * 4 + h4
                                recip = stat.tile([1, QC], BF16)
                                nc.vector.reciprocal(recip, pcs[jh][32:33, :])
                                pb = ps_s.tile([P, QC], FP32, tag="ps",
                                               name="pb")
                                nc.tensor.matmul(pb, ones1, recip,
                                                 start=True, stop=True)
                                recip_bc = work.tile([P, QC], BF16,
                                                     tag="recipbc")
                                nc.vector.tensor_copy(recip_bc, pb)
                                nc.vector.tensor_mul(
                                    ctxT[32 * h4:32 * (h4 + 1), g,
                                         q0:q0 + QC],
                                    pcs[jh][0:32, :], recip_bc[0:32, :])
                                rbN = recip_bc[:, None, :].broadcast_to(
                                    (P, NT, QC))
                                if "norm" not in ABL:
                                    nc.vector.tensor_mul(
                                        exp_sb[:, :, jh, :],
                                        exp_sb[:, :, jh, :], rbN)
                                dma_args = (
                                    attn_out[h, :, q0:q0 + QC].rearrange(
                                        "(kt p) q -> p kt q", p=P),
                                    exp_sb[:, :, jh, :])
                                if "attndma" in ABL:
                                    pass
                                elif jh:
                                    nc.sync.dma_start(*dma_args)
                                else:
                                    deferred_dmas.append(dma_args)
                for args in deferred_prev + deferred_dmas:
                    nc.scalar.dma_start(*args)

        # ---- phase F: att_out = ctx @ Wo, transposed to token-major ----
        with tc.tile_pool(name="mlpbuf", bufs=1) as mlpbuf, \
             tc.tile_pool(name="ps_sm2", bufs=3, space="PSUM") as ps_sm2:
            hT = mlpbuf.tile([P, KD, TQ], BF16)
            mlp1gT = mlpbuf.tile([P, DH // P, TQ], BF16)

            for m in range(KD):
                for qc in range(2):
                    pm = ps_sm2.tile([P, 512], FP32, tag="ps")
                    for g in range(2):
                        nc.tensor.matmul(
                            pm, wo_sb[:, g, m * P:(m + 1) * P],
                            ctxT[:, g, qc * 512:(qc + 1) * 512],
                            start=(g == 0), stop=(g == 1))
                    ao = work.tile([P, 512], BF16)
                    nc.scalar.copy(ao, pm)
                    for j in range(4):
                        jt = qc * 4 + j
                        nc.sync.dma_start_transpose(
                            attout[:, jt, m * P:(m + 1) * P],
                            ao[:, j * P:(j + 1) * P])

            # ---- phase G (staged): y = LN(attout + xx), x1 = x + y, h = LN2 ----
            sball = mlpbuf.tile([P, NTQ, D], FP32)
            xoall = mlpbuf.tile([P, NTQ, D], FP32)
            mv2 = mlpbuf.tile([P, NTQ, 2], FP32)
            rs2 = mlpbuf.tile([P, NTQ], FP32)
            for jt in range(NTQ):
                nc.scalar.dma_start(xoall[:, jt, :], xb[jt * P:(jt + 1) * P, :])
            for jt in range(NTQ):
                nc.vector.tensor_add(sball[:, jt, :], attout[:, jt, :],
                                     xx_mine[:, jt, :])
                st = stat.tile([P, 6], FP32)
                nc.vector.bn_stats(st, sball[:, jt, :])
                nc.vector.bn_aggr(mv2[:, jt, :], st)
            for jt in range(NTQ):
                nc.scalar.activation(rs2[:, jt:jt + 1], mv2[:, jt, 1:2],
                                     AF.Sqrt, bias=ebias, scale=1.0)
            nc.vector.reciprocal(rs2, rs2)
            for jt in range(NTQ):
                y = work.tile([P, D], FP32)
                nc.vector.tensor_scalar(
                    out=y, in0=sball[:, jt, :], scalar1=mv2[:, jt, 0:1],
                    scalar2=rs2[:, jt:jt + 1],
                    op0=mybir.AluOpType.subtract, op1=mybir.AluOpType.mult)
                nc.vector.tensor_add(x1[:, jt, :], y, xoall[:, jt, :])
                st = stat.tile([P, 6], FP32)
                nc.vector.bn_stats(st, x1[:, jt, :])
                nc.vector.bn_aggr(mv2[:, jt, :], st)
            for jt in range(NTQ):
                nc.scalar.activation(rs2[:, jt:jt + 1], mv2[:, jt, 1:2],
                                     AF.Sqrt, bias=ebias, scale=1.0)
            nc.vector.reciprocal(rs2, rs2)
            for jt in range(NTQ):
                ht = work.tile([P, D], BF16)
                nc.vector.tensor_scalar(
                    out=ht, in0=x1[:, jt, :], scalar1=mv2[:, jt, 0:1],
                    scalar2=rs2[:, jt:jt + 1],
                    op0=mybir.AluOpType.subtract, op1=mybir.AluOpType.mult)
                nc.vector.tensor_mul(ht, ht, ln2w_sb)
                nc.vector.tensor_add(ht, ht, ln2b_sb)
                for kd in range(KD):
                    pt2 = ps_sm2.tile([P, 512], BF16, tag="pt2",
                                      name="pt2")[:, 0:P]
                    nc.tensor.transpose(pt2, ht[:, kd * P:(kd + 1) * P], id_bf)
                    nc.scalar.copy(hT[:, kd, jt * P:(jt + 1) * P], pt2)

            # ---- phase H: MLP ----
            for m in range(DH // P):
                for qc in range(2):
                    pm = ps_sm2.tile([P, 512], FP32, tag="ps")
                    for kd in range(KD):
                        nc.tensor.matmul(
                            pm, w1_sb[:, kd, m * P:(m + 1) * P],
                            hT[:, kd, qc * 512:(qc + 1) * 512],
                            start=(kd == 0), stop=(kd == KD - 1))
                    nc.scalar.activation(
                        mlp1gT[:, m, qc * 512:(qc + 1) * 512], pm, AF.Gelu,
                        bias=b1_sb[:, m:m + 1], scale=1.0)
            for jt in range(NTQ):
                pm = ps_sm2.tile([P, 512], FP32, tag="ps", name="pm2")[:, 0:D]
                for m in range(DH // P):
                    nc.tensor.matmul(
                        pm, mlp1gT[:, m, jt * P:(jt + 1) * P], w2_sb[:, m, :],
                        start=(m == 0), stop=(m == DH // P - 1))
                ysb = work.tile([P, D], FP32)
                nc.vector.tensor_add(ysb, pm, b2_sb)
                nc.vector.tensor_add(ysb, ysb, x1[:, jt, :])
                nc.sync.dma_start(y_out[jt * P:(jt + 1) * P, :], ysb)

    nc.compile()
    return nc


_CACHE = {}


def kernel(**inputs):
    import ml_dtypes
    x = np.asarray(inputs["x"], np.float32)
    mask = np.asarray(inputs["attn_mask"])
    sc = 1.0 / math.sqrt(DK)

    bf = ml_dtypes.bfloat16
    wq_s = (np.asarray(inputs["Wq"], np.float32) * sc).astype(bf)
    wk = np.asarray(inputs["Wk"], np.float32).astype(bf)
    wv = np.asarray(inputs["Wv"], np.float32)
    wvaug = np.zeros((D, H * 33), np.float32)
    for h in range(H):
        wvaug[:, h * 33:h * 33 + 32] = wv[:, h * 32:(h + 1) * 32]
    wvaug = wvaug.astype(bf)
    wo = np.asarray(inputs["Wo"], np.float32).astype(bf)
    w1 = np.asarray(inputs["W1"], np.float32).astype(bf)
    w2 = np.asarray(inputs["W2"], np.float32).astype(bf)
    b1t = np.ascontiguousarray(
        np.asarray(inputs["b1"], np.float32).reshape(DH // P, P).T)
    b2b = np.broadcast_to(np.asarray(inputs["b2"], np.float32), (P, D)).copy()
    ln1wb = np.broadcast_to(np.asarray(inputs["ln1_w"], np.float32), (P, D)).copy()
    ln1bb = np.broadcast_to(np.asarray(inputs["ln1_b"], np.float32), (P, D)).copy()
    ln2wb = np.broadcast_to(np.asarray(inputs["ln2_w"], np.float32), (P, D)).copy()
    ln2bb = np.broadcast_to(np.asarray(inputs["ln2_b"], np.float32), (P, D)).copy()

    shared = dict(wq=wq_s, wk=wk, wvaug=wvaug, wo=wo, w1=w1, w2=w2, b1t=b1t,
                  b2b=b2b, ln1w=ln1wb, ln1b=ln1bb, ln2w=ln2wb, ln2b=ln2bb)

    in_maps = []
    for c in range(8):
        b, s = c // 2, c % 2
        perm = np.r_[s * TQ:(s + 1) * TQ, (1 - s) * TQ:(2 - s) * TQ]
        xbp = np.ascontiguousarray(x[b][perm])
        mk = np.ascontiguousarray(
            mask[b][perm[:TQ]][:, perm].T.astype(np.uint8))
        in_maps.append(dict(xb=xbp, masku=mk, **shared))

    if "nc" not in _CACHE:
        _CACHE["nc"] = build_graph()
    nc = _CACHE["nc"]

    from concourse.bass_utils import run_bass_kernel_spmd
    res = run_bass_kernel_spmd(nc, in_maps, core_ids=list(range(8)),
                               trace=bool(int(os.environ.get("BASS_TRACE", "0"))))
    _CACHE["last_result"] = res

    out_y = np.empty((B, T, D), np.float32)
    out_attn = np.empty((B, H, T, T), np.float32)
    for c in range(8):
        b, s = c // 2, c % 2
        perm = np.r_[s * TQ:(s + 1) * TQ, (1 - s) * TQ:(2 - s) * TQ]
        rr = res.results[c]
        out_y[b, s * TQ:(s + 1) * TQ] = rr["y_out"]
        a = np.asarray(rr["attn_out"]).astype(np.float32)  # [H, k_perm, q]
        out_attn[b, :, s * TQ:(s + 1) * TQ, :] = a.transpose(0, 2, 1)[:, :, perm]
    return (out_y, out_attn)


if __name__ == "__main__":
    nc = build_graph()
    print("graph built ok")


# revision 53
# speedup vs baseline: 1.0103x; 1.0103x over previous
"""Trainium2 Bass kernel for a dense transformer block (B=4, T=2048, D=256, H=8).

Sharding: 8 cores, zero collectives. Core c owns batch b=c//2 and query-half
s=c%2. Tokens are host-reordered per core so the core's 1024 query rows are
always tokens 0-1023 (uniform SPMD graph). Each core computes LN1+QKV over all
2048 tokens of its batch (k/v need them), attention for all 8 heads over its
1024 query rows, and the Wo/LN/MLP tail for its rows.

Attention is computed in [k, q] orientation (scores^T) so the softmax
normalizer and ctx both come from PE matmuls against v augmented with a ones
column; the attn output is stored [h, k, q] per core and transposed on the
host during unshard. attn values travel as bf16 (tol 2e-2), the -64*mask bias
is added into PSUM via an fp8 identity matmul.
"""

import math
import os
import sys
from contextlib import ExitStack

import numpy as np

sys.path.insert(0, "/opt/trn_rl_repo")

import concourse.bass as bass
import concourse.bacc as bacc
import concourse.tile as tile
from concourse import mybir
from concourse.masks import make_identity

FP32 = mybir.dt.float32
FP32R = mybir.dt.float32r
BF16 = mybir.dt.bfloat16
FP8 = mybir.dt.float8e5
U8 = mybir.dt.uint8

B, T, D, H, DK = 4, 2048, 256, 8, 32
TQ = 1024          # query rows per core
DH = 4 * D         # 1024 mlp hidden
P = 128
NT = T // P        # 16 token tiles
NTQ = TQ // P      # 8 my-token tiles
KD = D // P        # 2 contraction tiles over D
EPS = 1e-5
MASK_NEG = -64.0   # exp(-64) ~ 1.6e-28 ~ 0; exactly representable in fp8e5
AF = mybir.ActivationFunctionType
ABL = set(os.environ.get("ABLATE", "").split(","))


def r(ap):
    return ap.bitcast(FP32R)


def build_graph():
    nc = bacc.Bacc(None, target_bir_lowering=False)

    xb = nc.declare_dram_parameter("xb", [T, D], FP32, isOutput=False)
    masku = nc.declare_dram_parameter("masku", [T, TQ], U8, isOutput=False)
    wq = nc.declare_dram_parameter("wq", [D, D], BF16, isOutput=False)
    wk = nc.declare_dram_parameter("wk", [D, D], BF16, isOutput=False)
    wvaug = nc.declare_dram_parameter("wvaug", [D, H * 33], BF16, isOutput=False)
    wo = nc.declare_dram_parameter("wo", [D, D], BF16, isOutput=False)
    w1 = nc.declare_dram_parameter("w1", [D, DH], BF16, isOutput=False)
    w2 = nc.declare_dram_parameter("w2", [DH, D], BF16, isOutput=False)
    b1t = nc.declare_dram_parameter("b1t", [P, DH // P], FP32, isOutput=False)
    b2b = nc.declare_dram_parameter("b2b", [P, D], FP32, isOutput=False)
    ln1w = nc.declare_dram_parameter("ln1w", [P, D], FP32, isOutput=False)
    ln1b = nc.declare_dram_parameter("ln1b", [P, D], FP32, isOutput=False)
    ln2w = nc.declare_dram_parameter("ln2w", [P, D], FP32, isOutput=False)
    ln2b = nc.declare_dram_parameter("ln2b", [P, D], FP32, isOutput=False)

    attn_out = nc.declare_dram_parameter("attn_out", [H, T, TQ], BF16, isOutput=True)
    y_out = nc.declare_dram_parameter("y_out", [TQ, D], FP32, isOutput=True)

    with tile.TileContext(nc) as tc, ExitStack() as ctx:
        ctx.enter_context(nc.allow_low_precision(
            reason="bf16 attention/mlp path fits the 2e-2 tolerance"))
        const = ctx.enter_context(tc.tile_pool(name="const", bufs=1))
        resid = ctx.enter_context(tc.tile_pool(name="resid", bufs=1))
        work = ctx.enter_context(tc.tile_pool(name="work", bufs=4))
        stat = ctx.enter_context(tc.tile_pool(name="stat", bufs=8))

        # ---- constants into SBUF ----
        id_f32 = const.tile([P, P], FP32)
        make_identity(nc, id_f32)
        id_f8 = const.tile([P, P], FP8)
        make_identity(nc, id_f8)
        zbias = const.tile([P, 1], FP32)
        nc.vector.memset(zbias, 0.0)
        ebias = const.tile([P, 1], FP32)
        nc.vector.memset(ebias, EPS)
        ones1 = const.tile([1, P], BF16)
        nc.vector.memset(ones1, 1.0)
        id_bf = const.tile([P, P], BF16)
        make_identity(nc, id_bf)

        wq_sb = const.tile([P, KD, D], BF16)
        wk_sb = const.tile([P, KD, D], BF16)
        wva_sb = const.tile([P, KD, H * 33], BF16)
        wo_sb = const.tile([P, KD, D], BF16)
        w1_sb = const.tile([P, KD, DH], BF16)
        for kd in range(KD):
            nc.scalar.dma_start(wq_sb[:, kd, :], wq[kd * P:(kd + 1) * P, :])
            nc.scalar.dma_start(wk_sb[:, kd, :], wk[kd * P:(kd + 1) * P, :])
            nc.scalar.dma_start(wva_sb[:, kd, :], wvaug[kd * P:(kd + 1) * P, :])
            nc.scalar.dma_start(wo_sb[:, kd, :], wo[kd * P:(kd + 1) * P, :])
            nc.scalar.dma_start(w1_sb[:, kd, :], w1[kd * P:(kd + 1) * P, :])
        w2_sb = const.tile([P, DH // P, D], BF16)
        for m in range(DH // P):
            nc.scalar.dma_start(w2_sb[:, m, :], w2[m * P:(m + 1) * P, :])
        b1_sb = const.tile([P, DH // P], FP32)
        nc.sync.dma_start(b1_sb, b1t[:])
        b2_sb = const.tile([P, D], FP32)
        nc.sync.dma_start(b2_sb, b2b[:])
        ln1w_sb = const.tile([P, D], FP32)
        nc.sync.dma_start(ln1w_sb, ln1w[:])
        ln1b_sb = const.tile([P, D], FP32)
        nc.sync.dma_start(ln1b_sb, ln1b[:])
        ln2w_sb = const.tile([P, D], FP32)
        nc.sync.dma_start(ln2w_sb, ln2w[:])
        ln2b_sb = const.tile([P, D], FP32)
        nc.sync.dma_start(ln2b_sb, ln2b[:])

        # ---- residual-stream tiles (live across phases) ----
        xx_mine = resid.tile([P, NTQ, D], BF16)    # LN1 rows 0-1023
        x1 = resid.tile([P, NTQ, D], FP32)
        ctxT = resid.tile([P, 2, TQ], BF16)        # [4h*32, g, q]
        attout = resid.tile([P, NTQ, D], BF16)     # ctx @ Wo, token-major

        def layernorm_tile(dst, src, w_sb=None, b_sb=None):
            """dst = LN(src) over free dim (256), optional affine."""
            st = stat.tile([P, 6], FP32)
            nc.vector.bn_stats(st, src)
            mv = stat.tile([P, 2], FP32)
            nc.vector.bn_aggr(mv, st)
            rstd = stat.tile([P, 1], FP32)
            nc.scalar.activation(rstd, mv[:, 1:2], AF.Sqrt, bias=ebias, scale=1.0)
            nc.vector.reciprocal(rstd, rstd)
            nc.vector.tensor_scalar(
                out=dst, in0=src, scalar1=mv[:, 0:1], scalar2=rstd,
                op0=mybir.AluOpType.subtract, op1=mybir.AluOpType.mult)
            if w_sb is not None:
                nc.vector.tensor_mul(dst, dst, w_sb)
                nc.vector.tensor_add(dst, dst, b_sb)

        with tc.tile_pool(name="attnbuf", bufs=1) as attnbuf:
            kT4 = attnbuf.tile([P, 2, T], BF16)      # [4h*32 dk, g, k]
            qT4 = attnbuf.tile([P, 2, TQ], BF16)
            vaug = attnbuf.tile([P, NT, H * 33], BF16)
            maskneg = attnbuf.tile([P, NT, TQ], FP8)

            # ---- phase A-D: LN1, transpose, QKV projections ----
            with tc.tile_pool(name="pre", bufs=1) as pre, \
                 tc.tile_pool(name="ps_sm", bufs=4, space="PSUM") as ps_sm:
                xxT = pre.tile([P, KD, T], BF16)
                xbig = pre.tile([P, NT, D], FP32)
                mvall = pre.tile([P, NT, 2], FP32)
                rsall = pre.tile([P, NT], FP32)
                for blk in range(2):
                    jlo, jhi = blk * NT // 2, (blk + 1) * NT // 2
                    for jt in range(jlo, jhi):
                        nc.sync.dma_start(xbig[:, jt, :],
                                          xb[jt * P:(jt + 1) * P, :])
                        st = stat.tile([P, 6], FP32)
                        nc.vector.bn_stats(st, xbig[:, jt, :])
                        nc.vector.bn_aggr(mvall[:, jt, :], st)
                    for jt in range(jlo, jhi):
                        nc.scalar.activation(rsall[:, jt:jt + 1],
                                             mvall[:, jt, 1:2],
                                             AF.Sqrt, bias=ebias, scale=1.0)
                    nc.vector.reciprocal(rsall[:, jlo:jhi], rsall[:, jlo:jhi])
                    for jt in range(jlo, jhi):
                        xxt = work.tile([P, D], BF16)
                        nc.vector.tensor_scalar(
                            out=xxt, in0=xbig[:, jt, :],
                            scalar1=mvall[:, jt, 0:1],
                            scalar2=rsall[:, jt:jt + 1],
                            op0=mybir.AluOpType.subtract,
                            op1=mybir.AluOpType.mult)
                        nc.vector.tensor_mul(xxt, xxt, ln1w_sb)
                        nc.vector.tensor_add(xxt, xxt, ln1b_sb)
                        if jt < NTQ:
                            nc.gpsimd.tensor_copy(xx_mine[:, jt, :], xxt)
                        for kd in range(KD):
                            pt = ps_sm.tile([P, 512], BF16, tag="pt",
                                            name="pt")[:, 0:P]
                            nc.tensor.transpose(
                                pt, xxt[:, kd * P:(kd + 1) * P], id_bf)
                            nc.scalar.copy(xxT[:, kd, jt * P:(jt + 1) * P], pt)

                # qT4/kT4: [4h-group g] rows = wq columns g*128..g*128+127
                for g in range(2):
                    for qc in range(2):
                        pm = ps_sm.tile([P, 512], FP32, tag="ps")
                        for kd in range(KD):
                            nc.tensor.matmul(
                                pm, wq_sb[:, kd, g * P:(g + 1) * P],
                                xxT[:, kd, qc * 512:(qc + 1) * 512],
                                start=(kd == 0), stop=(kd == KD - 1))
                        nc.scalar.copy(qT4[:, g, qc * 512:(qc + 1) * 512], pm)
                    for kc in range(4):
                        pm = ps_sm.tile([P, 512], FP32, tag="ps")
                        for kd in range(KD):
                            nc.tensor.matmul(
                                pm, wk_sb[:, kd, g * P:(g + 1) * P],
                                xxT[:, kd, kc * 512:(kc + 1) * 512],
                                start=(kd == 0), stop=(kd == KD - 1))
                        nc.scalar.copy(kT4[:, g, kc * 512:(kc + 1) * 512], pm)

                # v_aug token-major, bf16, with ones columns at 32::33
                for jt in range(NT):
                    pm = ps_sm.tile([P, 512], FP32, tag="ps", name="pmv")[:, 0:H * 33]
                    for kd in range(KD):
                        nc.tensor.matmul(
                            pm, xxT[:, kd, jt * P:(jt + 1) * P],
                            wva_sb[:, kd, :],
                            start=(kd == 0), stop=(kd == KD - 1))
                    nc.scalar.copy(vaug[:, jt, :], pm)
                    nc.vector.memset(vaug[:, jt, 32::33], 1.0)

                for jt in range(NT):
                    mu = work.tile([P, TQ], U8, tag="masku")
                    nc.scalar.dma_start(mu, masku[jt * P:(jt + 1) * P, :])
                    nc.gpsimd.tensor_scalar_mul(maskneg[:, jt, :], mu, MASK_NEG)

            # ---- phase E: attention ----
            with tc.tile_pool(name="expbuf", bufs=1) as expbuf, \
                 tc.tile_pool(name="ps_s", bufs=6, space="PSUM") as ps_s, \
                 tc.tile_pool(name="ps_ctx", bufs=2, space="PSUM") as ps_ctx:
                QC = 512
                exp_bufs = [
                    expbuf.tile([P, NT, 2, QC], BF16, name=f"exp_sb{i}")
                    for i in range(3)
                ]
                it = 0
                deferred_dmas = []
                deferred_prev = []
                for g in range(2):
                    for qc in range(2):
                        q0 = qc * QC
                        for hp in range(2):
                            exp_sb = exp_bufs[it % 3]
                            it += 1
                            for args in deferred_prev:
                                nc.scalar.dma_start(*args)
                            deferred_prev = deferred_dmas
                            deferred_dmas = []
                            pcs = []
                            for jh in range(2):
                                pc = ps_ctx.tile([33, QC], FP32, tag="pc")
                                pcs.append(pc)
                            for kt in range(NT):
                                for jh in range(2):
                                    h4 = 2 * hp + jh
                                    h = g * 4 + h4
                                    ps1 = ps_s.tile([P, QC], FP32, tag="ps")
                                    nc.tensor.matmul(
                                        ps1,
                                        kT4[32 * h4:32 * (h4 + 1), g,
                                            kt * P:(kt + 1) * P],
                                        qT4[32 * h4:32 * (h4 + 1), g,
                                            q0:q0 + QC],
                                        start=True, stop=False,
                                        tile_position=(32 * h4, 0),
                                        skip_group_check=True)
                                    nc.tensor.matmul(
                                        ps1, id_f8,
                                        maskneg[:, kt, q0:q0 + QC],
                                        start=False, stop=True,
                                        skip_group_check=True)
                                    nc.scalar.activation(
                                        exp_sb[:, kt, jh, :],
                                        ps1, AF.Exp, bias=zbias, scale=1.0)
                                    nc.tensor.matmul(
                                        pcs[jh],
                                        vaug[:, kt, h * 33:(h + 1) * 33],
                                        exp_sb[:, kt, jh, :],
                                        start=(kt == 0),
                                        stop=(kt == NT - 1))
                            for jh in range(2):
                                h4 = 2 * hp + jh
                                h = g * 4 + h4
                                recip = stat.tile([1, QC], BF16)
                                nc.vector.reciprocal(recip, pcs[jh][32:33, :])
                                pb = ps_s.tile([P, QC], FP32, tag="ps",
                                               name="pb")
                                nc.tensor.matmul(pb, ones1, recip,
                                                 start=True, stop=True)
                                recip_bc = work.tile([P, QC], BF16,
                                                     tag="recipbc")
                                nc.vector.tensor_copy(recip_bc, pb)
                                nc.vector.tensor_mul(
                                    ctxT[32 * h4:32 * (h4 + 1), g,
                                         q0:q0 + QC],
                                    pcs[jh][0:32, :], recip_bc[0:32, :])
                                rbN = recip_bc[:, None, :].broadcast_to(
                                    (P, NT, QC))
                                if "norm" not in ABL:
                                    nc.vector.tensor_mul(
                                        exp_sb[:, :, jh, :],
                                        exp_sb[:, :, jh, :], rbN)
                                dma_args = (
                                    attn_out[h, :, q0:q0 + QC].rearrange(
                                        "(kt p) q -> p kt q", p=P),
                                    exp_sb[:, :, jh, :])
                                if "attndma" in ABL:
                                    pass
                                elif jh:
                                    nc.sync.dma_start(*dma_args)
                                else:
                                    deferred_dmas.append(dma_args)
                for args in deferred_prev + deferred_dmas:
                    nc.scalar.dma_start(*args)

        # ---- phase F: att_out = ctx @ Wo, transposed to token-major ----
        with tc.tile_pool(name="mlpbuf", bufs=1) as mlpbuf, \
             tc.tile_pool(name="ps_sm2", bufs=3, space="PSUM") as ps_sm2:
            hT = mlpbuf.tile([P, KD, TQ], BF16)
            mlp1gT = mlpbuf.tile([P, DH // P, TQ], BF16)

            for m in range(KD):
                for qc in range(2):
                    pm = ps_sm2.tile([P, 512], FP32, tag="ps")
                    for g in range(2):
                        nc.tensor.matmul(
                            pm, wo_sb[:, g, m * P:(m + 1) * P],
                            ctxT[:, g, qc * 512:(qc + 1) * 512],
                            start=(g == 0), stop=(g == 1))
                    ao = work.tile([P, 512], BF16)
                    nc.scalar.copy(ao, pm)
                    for j in range(4):
                        jt = qc * 4 + j
                        nc.scalar.dma_start_transpose(
                            attout[:, jt, m * P:(m + 1) * P],
                            ao[:, j * P:(j + 1) * P])

            # ---- phase G (staged): y = LN(attout + xx), x1 = x + y, h = LN2 ----
            sball = mlpbuf.tile([P, NTQ, D], FP32)
            xoall = mlpbuf.tile([P, NTQ, D], FP32)
            mv2 = mlpbuf.tile([P, NTQ, 2], FP32)
            rs2 = mlpbuf.tile([P, NTQ], FP32)
            for jt in range(NTQ):
                nc.scalar.dma_start(xoall[:, jt, :], xb[jt * P:(jt + 1) * P, :])
            for jt in range(NTQ):
                nc.gpsimd.tensor_add(sball[:, jt, :], attout[:, jt, :],
                                     xx_mine[:, jt, :])
                st = stat.tile([P, 6], FP32)
                nc.vector.bn_stats(st, sball[:, jt, :])
                nc.vector.bn_aggr(mv2[:, jt, :], st)
            for jt in range(NTQ):
                nc.scalar.activation(rs2[:, jt:jt + 1], mv2[:, jt, 1:2],
                                     AF.Sqrt, bias=ebias, scale=1.0)
            nc.vector.reciprocal(rs2, rs2)
            for jt in range(NTQ):
                y = work.tile([P, D], FP32)
                nc.vector.tensor_scalar(
                    out=y, in0=sball[:, jt, :], scalar1=mv2[:, jt, 0:1],
                    scalar2=rs2[:, jt:jt + 1],
                    op0=mybir.AluOpType.subtract, op1=mybir.AluOpType.mult)
                nc.gpsimd.tensor_add(x1[:, jt, :], y, xoall[:, jt, :])
                st = stat.tile([P, 6], FP32)
                nc.vector.bn_stats(st, x1[:, jt, :])
                nc.vector.bn_aggr(mv2[:, jt, :], st)
            for jt in range(NTQ):
                nc.scalar.activation(rs2[:, jt:jt + 1], mv2[:, jt, 1:2],
                                     AF.Sqrt, bias=ebias, scale=1.0)
            nc.vector.reciprocal(rs2, rs2)
            for jt in range(NTQ):
                ht = work.tile([P, D], BF16)
                nc.vector.tensor_scalar(
                    out=ht, in0=x1[:, jt, :], scalar1=mv2[:, jt, 0:1],
                    scalar2=rs2[:, jt:jt + 1],
                    op0=mybir.AluOpType.subtract, op1=mybir.AluOpType.mult)
                nc.vector.tensor_mul(ht, ht, ln2w_sb)
                nc.vector.tensor_add(ht, ht, ln2b_sb)
                for kd in range(KD):
                    pt2 = ps_sm2.tile([P, 512], BF16, tag="pt2",
                                      name="pt2")[:, 0:P]
                    nc.tensor.transpose(pt2, ht[:, kd * P:(kd + 1) * P], id_bf)
                    nc.scalar.copy(hT[:, kd, jt * P:(jt + 1) * P], pt2)

            # ---- phase H: MLP ----
            for m in range(DH // P):
                for qc in range(2):
                    pm = ps_sm2.tile([P, 512], FP32, tag="ps")
                    for kd in range(KD):
                        nc.tensor.matmul(
                            pm, w1_sb[:, kd, m * P:(m + 1) * P],
                            hT[:, kd, qc * 512:(qc + 1) * 512],
                            start=(kd == 0), stop=(kd == KD - 1))
                    nc.scalar.activation(
                        mlp1gT[:, m, qc * 512:(qc + 1) * 512], pm, AF.Gelu,
                        bias=b1_sb[:, m:m + 1], scale=1.0)
            for jt in range(NTQ):
                pm = ps_sm2.tile([P, 512], FP32, tag="ps", name="pm2")[:, 0:D]
                for m in range(DH // P):
                    nc.tensor.matmul(
                        pm, mlp1gT[:, m, jt * P:(jt + 1) * P], w2_sb[:, m, :],
                        start=(m == 0), stop=(m == DH // P - 1))
                ysb = work.tile([P, D], FP32)
                nc.vector.tensor_add(ysb, pm, b2_sb)
                nc.vector.tensor_add(ysb, ysb, x1[:, jt, :])
                nc.sync.dma_start(y_out[jt * P:(jt + 1) * P, :], ysb)

    nc.compile()
    return nc


_CACHE = {}


def kernel(**inputs):
    import ml_dtypes
    x = np.asarray(inputs["x"], np.float32)
    mask = np.asarray(inputs["attn_mask"])
    sc = 1.0 / math.sqrt(DK)

    bf = ml_dtypes.bfloat16
    wq_s = (np.asarray(inputs["Wq"], np.float32) * sc).astype(bf)
    wk = np.asarray(inputs["Wk"], np.float32).astype(bf)
    wv = np.asarray(inputs["Wv"], np.float32)
    wvaug = np.zeros((D, H * 33), np.float32)
    for h in range(H):
        wvaug[:, h * 33:h * 33 + 32] = wv[:, h * 32:(h + 1) * 32]
    wvaug = wvaug.astype(bf)
    wo = np.asarray(inputs["Wo"], np.float32).astype(bf)
    w1 = np.asarray(inputs["W1"], np.float32).astype(bf)
    w2 = np.asarray(inputs["W2"], np.float32).astype(bf)
    b1t = np.ascontiguousarray(
        np.asarray(inputs["b1"], np.float32).reshape(DH // P, P).T)
    b2b = np.broadcast_to(np.asarray(inputs["b2"], np.float32), (P, D)).copy()
    ln1wb = np.broadcast_to(np.asarray(inputs["ln1_w"], np.float32), (P, D)).copy()
    ln1bb = np.broadcast_to(np.asarray(inputs["ln1_b"], np.float32), (P, D)).copy()
    ln2wb = np.broadcast_to(np.asarray(inputs["ln2_w"], np.float32), (P, D)).copy()
    ln2bb = np.broadcast_to(np.asarray(inputs["ln2_b"], np.float32), (P, D)).copy()

    shared = dict(wq=wq_s, wk=wk, wvaug=wvaug, wo=wo, w1=w1, w2=w2, b1t=b1t,
                  b2b=b2b, ln1w=ln1wb, ln1b=ln1bb, ln2w=ln2wb, ln2b=ln2bb)

    in_maps = []
    for c in range(8):
        b, s = c // 2, c % 2
        perm = np.r_[s * TQ:(s + 1) * TQ, (1 - s) * TQ:(2 - s) * TQ]
        xbp = np.ascontiguousarray(x[b][perm])
        mk = np.ascontiguousarray(
            mask[b][perm[:TQ]][:, perm].T.astype(np.uint8))
        in_maps.append(dict(xb=xbp, masku=mk, **shared))

    if "nc" not in _CACHE:
        _CACHE["nc"] = build_graph()
    nc = _CACHE["nc"]

    from concourse.bass_utils import run_bass_kernel_spmd
    res = run_bass_kernel_spmd(nc, in_maps, core_ids=list(range(8)),
                               trace=bool(int(os.environ.get("BASS_TRACE", "0"))))
    _CACHE["last_result"] = res

    out_y = np.empty((B, T, D), np.float32)
    out_attn = np.empty((B, H, T, T), np.float32)
    for c in range(8):
        b, s = c // 2, c % 2
        perm = np.r_[s * TQ:(s + 1) * TQ, (1 - s) * TQ:(2 - s) * TQ]
        rr = res.results[c]
        out_y[b, s * TQ:(s + 1) * TQ] = rr["y_out"]
        a = np.asarray(rr["attn_out"]).astype(np.float32)  # [H, k_perm, q]
        out_attn[b, :, s * TQ:(s + 1) * TQ, :] = a.transpose(0, 2, 1)[:, :, perm]
    return (out_y, out_attn)


if __name__ == "__main__":
    nc = build_graph()
    print("graph built ok")
